# revision 15
# baseline (speedup 1.0000x reference)
"""Causal self-attention (B=2, T=2048, C=1024, H=16) on 8 trn2 NeuronCores.

Sharding: data-parallel over B (2) x tensor-parallel over head groups (4
groups of 4 heads).  core c -> batch c//4, head group c%4.  Each core
computes its 4 heads' qkv projection, attention, and the partial c_proj
contribution; the host sums the 4 tensor-parallel partials per batch
(the "all-reduce") and adds b_proj.

All matmuls run in bf16 (fp32r sustained at full rate trips the power
throttler, clamping the PE to 1.2 GHz).  Phases are interleaved per
t-block (qkv -> attention -> normalize -> c_proj) so the PE streams
back-to-back work; softmax exp is batched in [128,1024] pairs to
amortize ACT's per-instruction overhead; the softmax reciprocal runs on
the DVE.
"""

import sys
from contextlib import ExitStack

for _p in ("/opt/trn_rl_repo",):
    if _p not in sys.path:
        sys.path.insert(0, _p)

import ml_dtypes
import numpy as np

import concourse.bass as bass
import concourse.tile as tile
from concourse import mybir
from concourse.bass_utils import run_bass_kernel_spmd

F32 = mybir.dt.float32
F32R = mybir.dt.float32r
BF16 = mybir.dt.bfloat16
EXP = mybir.ActivationFunctionType.Exp

B, T, C = 2, 2048, 1024
H, D = 16, 64          # total heads, head dim
HL = 4                 # heads per core (local)
N_CORES = 8
QB = 512               # q block width
NTT = T // 128         # 16 t-tiles
NTB = T // QB          # 4 t-blocks
NC_C = C // 128        # 8 contraction tiles over C


def _merge(a, b):
    for k, v in b.items():
        if a.get(k, -1) < v:
            a[k] = v


def _reduce_matmul_waits(nc):
    """Sound transitive reduction of Matmult sync waits.

    Walrus rejects self-loading matmuls with >1 sync wait (the LDWEIGHTS
    struct has one wait slot).  Tile emits per-proc-minimal waits but does not
    track cross-proc transitivity, so e.g. a matmul recycling a PSUM slot
    waits on both the old writer (PE) and the old reader (ACT) even though the
    reader's wait already implies the writer finished.  We compute guarantee
    vector clocks (sem -> min value) for every sem increment and drop Matmult
    waits that are implied by the instruction's queue dispatch knowledge plus
    its remaining waits.
    """
    import bass_rust
    DMA_OPS = {"InstDMACopy", "InstDMATranspose"}
    dispatch = {}    # queue -> clock known at sequencer dispatch point
    done_prev = {}   # queue -> completion clock of previous engine inst
    sem_cum = {}     # sem -> cumulative inc
    sem_hist = {}    # sem -> list[(cum, prefix-merged clock)]
    n_dropped = 0

    def clock_at(sem, v):
        for cum, snap in sem_hist.get(sem, ()):
            if cum >= v:
                return snap
        return {}

    insts = [ins for bb in nc.main_func.blocks for ins in bb.instructions]
    for ins in insts:
        si = ins.sync_info
        q = str(getattr(ins, "engine", "?"))
        opc = type(ins).__name__
        dq = dispatch.setdefault(q, {})
        waits = list(si.on_wait) if si is not None else []
        wclocks = []
        for w in waits:
            wc = dict(clock_at(w.ant_name, w.wait_value))
            if wc.get(w.ant_name, -1) < w.wait_value:
                wc[w.ant_name] = w.wait_value
            wclocks.append(wc)

        if len(waits) > 1:
            # For serially-executing engines (DVE drains its pipe per op; ACT
            # and GpSimd likewise retire in order), the previous same-queue
            # instruction has fully completed by the time this one executes,
            # so its completion clock joins the implication base.  PE overlaps
            # matmul drains, and DMA lanes are async, so they only get
            # sequencer dispatch knowledge.
            serial = opc not in DMA_OPS and not q.endswith("PE")
            keep = set(range(len(waits)))
            order = sorted(
                range(len(waits)),
                key=lambda k: 0 if not waits[k].ant_name.startswith("DMA") else 1,
            )
            for k in order:
                if len(keep) <= 1:
                    break
                base = dict(dq)
                if serial:
                    _merge(base, done_prev.get(q, {}))
                for k2 in keep:
                    if k2 != k:
                        _merge(base, wclocks[k2])
                w = waits[k]
                if base.get(w.ant_name, -1) >= w.wait_value:
                    keep.discard(k)
            if len(keep) < len(waits):
                n_dropped += len(waits) - len(keep)
                ins.sync_info = bass_rust.SyncInfo(
                    on_wait=[waits[k] for k in sorted(keep)],
                    on_update=list(si.on_update),
                )

        for wc in wclocks:
            _merge(dq, wc)

        comp = dict(dq)
        if opc not in DMA_OPS:
            _merge(comp, done_prev.get(q, {}))

        ups = list(si.on_update) if si is not None else []
        for u in ups:
            if u.update_mode != "sem-inc":
                continue
            cum = sem_cum.get(u.ant_name, 0) + u.update_value
            sem_cum[u.ant_name] = cum
            hist = sem_hist.setdefault(u.ant_name, [])
            snap = dict(hist[-1][1]) if hist else {}
            _merge(snap, comp)
            snap[u.ant_name] = cum
            hist.append((cum, snap))
        if opc not in DMA_OPS:
            comp2 = dict(comp)
            for u in ups:
                if u.update_mode == "sem-inc":
                    comp2[u.ant_name] = max(
                        comp2.get(u.ant_name, 0), sem_cum[u.ant_name])
            done_prev[q] = comp2

    bad = [
        (ins.name, [(w.ant_name, w.wait_value) for w in ins.sync_info.on_wait])
        for ins in insts
        if type(ins).__name__ == "InstMatmult"
        and ins.sync_info is not None and len(ins.sync_info.on_wait) > 1
    ]
    if bad:
        raise RuntimeError(f"{len(bad)} matmuls still have >1 wait: {bad[:8]}")

    # This walrus accepts at most ONE sync wait per instruction struct.
    # Matmuls are handled above; for everything else, hoist the extra waits
    # onto standalone single-wait Drain carriers on the same queue (the
    # sequencer executes them in order, so the semantics are unchanged).
    wid = 0
    for bb in nc.main_func.blocks:
        out_list = []
        changed = False
        for ins in bb.instructions:
            si = ins.sync_info
            if (si is not None and len(si.on_wait) > 1
                    and type(ins).__name__ != "InstMatmult"):
                waits = list(si.on_wait)
                for w in waits[:-1]:
                    d = mybir.InstDrain(name=f"WSPLIT-{wid}", ins=[], outs=[])
                    wid += 1
                    d.engine = ins.engine
                    d.sync_info = bass_rust.SyncInfo(on_wait=[w], on_update=[])
                    try:
                        nc.register_instruction(d)
                    except Exception:
                        pass
                    out_list.append(d)
                ins.sync_info = bass_rust.SyncInfo(
                    on_wait=[waits[-1]], on_update=list(si.on_update))
                changed = True
            out_list.append(ins)
        if changed:
            bb.instructions = out_list

    # This neuronxcc's walrus rejects the raw-ISA EVENT_SEMAPHORE_RANGE_CLEAR
    # Tile emits as end-of-program semaphore hygiene ("ISA wrong length").
    # It has no sync side effects and only matters for back-to-back reuse of
    # the semaphore window inside one program, so drop it.
    for bb in nc.main_func.blocks:
        kept = [i for i in bb.instructions
                if not (type(i).__name__ == "InstISA"
                        and getattr(i, "op_name", "") ==
                        "EVENT_SEMAPHORE_RANGE_CLEAR")]
        if len(kept) != len(bb.instructions):
            bb.instructions = kept


def _build_nc(with_bias: bool = False) -> bass.Bass:
    nc = bass.Bass()

    xt = nc.declare_dram_parameter("xt", [C, T], BF16, False)
    wqk = nc.declare_dram_parameter("wqk", [C, 512], BF16, False)
    bqk = nc.declare_dram_parameter("bqk", [1, 512], BF16, False) if with_bias else None
    wv = nc.declare_dram_parameter("wv", [C, 256], BF16, False)
    bv = nc.declare_dram_parameter("bv", [1, 256], BF16, False) if with_bias else None
    wp4 = nc.declare_dram_parameter("wp4", [64, HL, C], BF16, False)
    tri = nc.declare_dram_parameter("tri", [128, 128], F32, False)
    out = nc.declare_dram_parameter("out", [T, C], F32, True)

    with tile.TileContext(nc) as tc, ExitStack() as ctx:
        consts = ctx.enter_context(tc.tile_pool(name="consts", bufs=1))
        wpool = ctx.enter_context(tc.tile_pool(name="wpool", bufs=1))
        big = ctx.enter_context(tc.tile_pool(name="big", bufs=1))
        xtp = ctx.enter_context(tc.tile_pool(name="xtp", bufs=2))
        ptp = ctx.enter_context(tc.tile_pool(name="ptp", bufs=3))
        yjp = ctx.enter_context(tc.tile_pool(name="yjp", bufs=4))
        otp = ctx.enter_context(tc.tile_pool(name="otp", bufs=2))
        small = ctx.enter_context(tc.tile_pool(name="small", bufs=2))
        bigps = ctx.enter_context(tc.tile_pool(name="bigps", bufs=2, space="PSUM"))
        psyp = ctx.enter_context(tc.tile_pool(name="psyp", bufs=2, space="PSUM"))
        psbp = ctx.enter_context(tc.tile_pool(name="psbp", bufs=2, space="PSUM"))

        # ---- constants ----
        tri_sb = consts.tile([128, 128], F32)   # additive: 0 if kk<=cc else -1e30
        nc.sync.dma_start(out=tri_sb, in_=tri[:])
        ones_b = consts.tile([1, 512], BF16)
        warm = consts.tile([1, 8], F32)

        if with_bias:
            bqk_sb = consts.tile([1, 512], BF16)
            nc.sync.dma_start(out=bqk_sb, in_=bqk[:])
            bv_sb = consts.tile([1, 256], BF16)
            nc.sync.dma_start(out=bv_sb, in_=bv[:])

        # ---- weights ----
        wqk_sb = wpool.tile([128, NC_C, 512], BF16)
        nc.sync.dma_start(out=wqk_sb, in_=wqk[:].rearrange("(c p) n -> p c n", p=128))
        wv_sb = wpool.tile([128, NC_C, 256], BF16)
        nc.sync.dma_start(out=wv_sb, in_=wv[:].rearrange("(c p) n -> p c n", p=128))
        wp_sb = wpool.tile([64, HL, C], BF16)
        nc.sync.dma_start(out=wp_sb, in_=wp4[:])

        # ---- persistent activations ----
        qkT = big.tile([128, 4, T], BF16)    # [qk ct, t]: rows = [q01|q23|k01|k23]*64
        v_sb = big.tile([128, NTT, HL, 65], BF16)   # v natural + ones column
        sums2 = big.tile([1, NTB * HL * 512], F32)   # raw softmax denoms, seg j*4+h
        recs = big.tile([1, NTB * HL * 512], BF16)   # 1/denominator

        # all constant memsets on DVE (single-proc tick, observed once by PE)
        nc.vector.memset(v_sb[:, :, :, 64:65], 1.0)
        nc.vector.memset(ones_b, 1.0)
        nc.vector.memset(warm, 0.0)
        nc.scalar.activation(out=warm, in_=warm, func=EXP)  # pre-load exp table

        # Self-loading matmuls support a single sync wait, so every real
        # matmul must have at most one un-observed dependency.  Funnel all
        # input-DMA completions through DVE copies, then observe DVE once on
        # the PE via a tiny "touch" matmul.  Touches write the psb bank and
        # get a DVE reader so their slot release is a DVE-only event.
        scr = consts.tile([1, 8], F32)
        funnel = [wqk_sb[0:1, 0, 0:1], wv_sb[0:1, 0, 0:1],
                  wp_sb[0:1, 0, 0:1], tri_sb[0:1, 0:1]]
        if with_bias:
            funnel += [bqk_sb[:, 0:1], bv_sb[:, 0:1]]
        for k, t_ap in enumerate(funnel):
            nc.vector.tensor_copy(out=scr[:, k:k + 1], in_=t_ap)
        scr2 = consts.tile([1, 1], F32)
        nc.vector.tensor_copy(out=scr2, in_=scr[:, 0:1])

        def touch(src_ap):
            sc = psbp.tile([1, 1], F32, tag="psb", name="sc")
            nc.tensor.matmul(out=sc, lhsT=src_ap, rhs=src_ap, start=True, stop=True)
            scd = small.tile([1, 1], F32, tag="scd", name="scd")
            nc.vector.tensor_copy(out=scd, in_=sc)

        touch(scr2)

        # ---- emission units: attention pairs interleaved with qkv/proj ----
        def load_x(tb):
            ts = slice(tb * QB, (tb + 1) * QB)
            xt_tb = xtp.tile([128, NC_C, QB], BF16)
            nc.sync.dma_start(
                out=xt_tb, in_=xt[:, ts].rearrange("(c p) n -> p c n", p=128)
            )
            return xt_tb

        def qkv_units(tb, xt_tb):
            ts = slice(tb * QB, (tb + 1) * QB)

            def touch_unit():
                xt_t = xtp.tile([1, 1], F32, tag="xt_t", name="xt_t")
                nc.vector.tensor_copy(out=xt_t, in_=xt_tb[0:1, 0, 0:1])
                touch(xt_t)

            def cp_unit(cp):
                # qk projection for ct pair cp into one 2-bank psum tile
                ps = bigps.tile([128, 1024], F32, tag="s")
                for sub in range(2):
                    ct = 2 * cp + sub
                    sl = slice(sub * 512, (sub + 1) * 512)
                    for c in range(NC_C):
                        nc.tensor.matmul(
                            out=ps[:, sl],
                            lhsT=wqk_sb[:, c, ct * 128:(ct + 1) * 128],
                            rhs=xt_tb[:, c, :],
                            start=(c == 0),
                            stop=(not with_bias and c == NC_C - 1),
                        )
                    if with_bias:
                        nc.tensor.matmul(  # + bias (outer product with ones)
                            out=ps[:, sl],
                            lhsT=bqk_sb[:, ct * 128:(ct + 1) * 128],
                            rhs=ones_b[:, 0:QB],
                            start=False,
                            stop=True,
                        )
                nc.scalar.copy(
                    out=qkT[:, 2 * cp:2 * cp + 2, ts],
                    in_=ps[:].rearrange("p (c n) -> p c n", c=2),
                )

            def v_unit(half):
                # v projection for two t-tiles into a 1-bank psum tile
                ps = bigps.tile([128, 512], F32, tag="s", name="psv")
                for sub in range(2):
                    t4 = 2 * half + sub
                    sl = slice(sub * 256, (sub + 1) * 256)
                    for c in range(NC_C):
                        nc.tensor.matmul(
                            out=ps[:, sl],
                            lhsT=xt_tb[:, c, t4 * 128:(t4 + 1) * 128],
                            rhs=wv_sb[:, c, :],
                            start=(c == 0),
                            stop=(not with_bias and c == NC_C - 1),
                        )
                    if with_bias:
                        nc.tensor.matmul(
                            out=ps[:, sl],
                            lhsT=ones_b[:, 0:128],
                            rhs=bv_sb[:],
                            start=False,
                            stop=True,
                        )
                tt = tb * 4 + 2 * half
                nc.scalar.copy(
                    out=v_sb[:, tt:tt + 2, :, 0:64],
                    in_=ps[:].rearrange("p (t h d) -> p t h d", t=2, h=HL),
                )

            return [touch_unit,
                    lambda: cp_unit(0), lambda: cp_unit(1),
                    lambda: v_unit(0), lambda: v_unit(1)]

        def norm_broadcast(j, h, yjs):
            # psb = ones x (1/sums[j,h]); emitted at least one head late so
            # the PE FIFO never stalls on the DVE reciprocal chain.
            idx = (j * HL + h) * 512
            psb = psbp.tile([64, 512], F32, tag="psb", name="psb")
            nc.tensor.matmul(
                out=psb, lhsT=ones_b[:, 0:64],
                rhs=recs[:, idx:idx + 512], start=True, stop=True,
            )
            nc.vector.tensor_mul(out=yjs[h], in0=yjs[h], in1=psb)

        def attn(j, yjs, filler):
            # Interleave one filler unit (qkv of next block / proj of previous
            # block) between attention pairs: dense PE work covers the
            # S -> exp -> PV dependency gap so the HAM clock gate stays warm.
            qs = slice(j * QB, (j + 1) * QB)
            nk = 4 * j + 4
            npairs = HL * (nk // 2)
            interval = max(1, npairs // (len(filler) + 1))
            pair_no = 0
            for h in range(HL):
                ct_q, ct_k, base = h // 2, 2 + h // 2, (h % 2) * 64
                rows = slice(base, base + 64)
                psy = psyp.tile([65, 512], F32, tag="psy")
                for m in range(nk // 2):
                    ps = bigps.tile([128, 1024], F32, tag="s")
                    os = [max(0, 128 * (2 * m + sub - 4 * j)) for sub in range(2)]
                    diag = 2 * m >= 4 * j
                    for sub in range(2):
                        i = 2 * m + sub
                        # trim diagonal S tiles to the causal columns
                        nc.tensor.matmul(
                            out=ps[:, sub * 512 + os[sub]:(sub + 1) * 512],
                            lhsT=qkT[rows, ct_k, i * 128:(i + 1) * 128],
                            rhs=qkT[rows, ct_q, qs][:, os[sub]:512],
                            start=True,
                            stop=True,
                        )
                        if diag:
                            nc.vector.tensor_add(
                                out=ps[:, sub * 512 + os[sub]:
                                       sub * 512 + os[sub] + 128],
                                in0=ps[:, sub * 512 + os[sub]:
                                        sub * 512 + os[sub] + 128],
                                in1=tri_sb,
                            )
                    p_t = ptp.tile([128, 1024], BF16, tag="p")
                    if diag:
                        # split exp per sub-tile: psum cols below os are unwritten
                        for sub in range(2):
                            sl = slice(sub * 512 + os[sub], (sub + 1) * 512)
                            nc.scalar.activation(
                                out=p_t[:, sl], in_=ps[:, sl], func=EXP,
                                scale=0.125,
                            )
                    else:
                        nc.scalar.activation(
                            out=p_t, in_=ps, func=EXP, scale=0.125
                        )
                    if pair_no % interval == interval - 1 and filler:
                        filler.pop(0)()   # PE work while ACT computes the exp
                    pair_no += 1
                    for sub in range(2):
                        i = 2 * m + sub
                        o = os[sub]
                        nc.tensor.matmul(
                            out=psy[:, o:512],
                            lhsT=v_sb[:, i, h, :],
                            rhs=p_t[:, sub * 512 + o:(sub + 1) * 512],
                            start=(i == 0),
                            stop=(i == nk - 1),
                        )
                yj = yjp.tile([64, 512], BF16, tag=f"yj{h}", name=f"yj{h}")
                yjs[h] = yj
                idx = (j * HL + h) * 512
                nc.vector.tensor_copy(
                    out=sums2[:, idx:idx + 512], in_=psy[64:65, :]
                )
                with nc.allow_low_precision(reason="softmax 1/sum in bf16"):
                    nc.vector.reciprocal(
                        out=recs[:, idx:idx + 512], in_=sums2[:, idx:idx + 512]
                    )
                nc.vector.tensor_copy(out=yj, in_=psy[0:64, :])
                if h >= 1:
                    norm_broadcast(j, h - 1, yjs)
            while filler:
                filler.pop(0)()

        def proj_units(tb, yjs):
            def t4_unit(t4):
                pso = bigps.tile([128, 1024], F32, tag="s")
                for co in range(2):
                    for h in range(HL):
                        nc.tensor.matmul(
                            out=pso[:, co * 512:(co + 1) * 512],
                            lhsT=yjs[h][:, t4 * 128:(t4 + 1) * 128],
                            rhs=wp_sb[:, h, co * 512:(co + 1) * 512],
                            start=(h == 0),
                            stop=(h == HL - 1),
                        )
                ot = otp.tile([128, C], F32, tag="ot")
                nc.scalar.copy(out=ot[:, 0:512], in_=pso[:, 0:512])
                nc.vector.tensor_copy(out=ot[:, 512:1024], in_=pso[:, 512:1024])
                t0 = tb * QB + t4 * 128
                nc.sync.dma_start(out=out[t0:t0 + 128, :], in_=ot)

            return [lambda t4=t4: t4_unit(t4) for t4 in range(4)]

        # Filler schedule (proj deferred up to two blocks so the long late
        # attention phases get enough dense PE work):
        #   attn(0): qkv(1)   attn(1): qkv(2)   attn(2): qkv(3)+proj(0)
        #   attn(3): proj(1)+proj(2)   tail: proj(3)
        xt_cur = load_x(0)
        for u in qkv_units(0, xt_cur):
            u()
        yjs_all = {}
        for tb in range(NTB):
            filler = []
            if tb + 1 < NTB:
                xt_nxt = load_x(tb + 1)
                filler += qkv_units(tb + 1, xt_nxt)
            if tb == 2:
                filler += proj_units(0, yjs_all[0])
            elif tb == 3:
                filler += proj_units(1, yjs_all[1])
                filler += proj_units(2, yjs_all[2])
            yjs = {}
            attn(tb, yjs, filler)
            norm_broadcast(tb, HL - 1, yjs)
            yjs_all[tb] = yjs
        for u in proj_units(NTB - 1, yjs_all[NTB - 1]):
            u()

    _reduce_matmul_waits(nc)
    return nc


_NC_CACHE = {}


def _get_nc(with_bias: bool = False):
    if with_bias not in _NC_CACHE:
        _NC_CACHE[with_bias] = _build_nc(with_bias)
    return _NC_CACHE[with_bias]


def make_in_maps(x, W_attn, b_attn, W_proj, b_proj):
    bf16 = ml_dtypes.bfloat16
    x = np.asarray(x, dtype=np.float32)
    W_attn = np.asarray(W_attn, dtype=np.float32)
    b_attn = np.asarray(b_attn, dtype=np.float32)
    W_proj = np.asarray(W_proj, dtype=np.float32)
    kk, cc = np.meshgrid(np.arange(128), np.arange(128), indexing="ij")
    tri = np.where(kk <= cc, 0.0, -1e30).astype(np.float32)
    in_maps = []
    for core in range(N_CORES):
        b, hg = core // 4, core % 4
        qc = slice(hg * 256, (hg + 1) * 256)
        kc = slice(C + hg * 256, C + (hg + 1) * 256)
        vc = slice(2 * C + hg * 256, 2 * C + (hg + 1) * 256)
        in_maps.append({
            "xt": np.ascontiguousarray(x[b].T).astype(bf16),
            "wqk": np.ascontiguousarray(
                np.concatenate([W_attn[:, qc], W_attn[:, kc]], axis=1)
            ).astype(bf16),
            "bqk": np.concatenate([b_attn[qc], b_attn[kc]]).reshape(1, 512)
            .astype(bf16),
            "wv": np.ascontiguousarray(W_attn[:, vc]).astype(bf16),
            "bv": b_attn[vc].reshape(1, 256).astype(bf16),
            "wp4": np.ascontiguousarray(
                W_proj[hg * 256:(hg + 1) * 256].reshape(HL, 64, C)
                .transpose(1, 0, 2)).astype(bf16),
            "tri": tri,
        })
    return in_maps


def gather(results, b_proj):
    b_proj = np.asarray(b_proj, dtype=np.float32)
    out = np.empty((B, T, C), dtype=np.float32)
    for b in range(B):
        acc = results[4 * b]["out"].astype(np.float32)
        for g in range(1, 4):
            acc = acc + results[4 * b + g]["out"]
        out[b] = acc + b_proj
    return out


def run(x, W_attn, b_attn, W_proj, b_proj, trace=False):
    with_bias = bool(np.any(np.asarray(b_attn)))
    nc = _get_nc(with_bias=with_bias)
    in_maps = make_in_maps(x, W_attn, b_attn, W_proj, b_proj)
    if not with_bias:
        for m in in_maps:
            m.pop("bqk", None)
            m.pop("bv", None)
    res = run_bass_kernel_spmd(nc, in_maps, list(range(N_CORES)), trace=trace)
    return gather(res.results, b_proj), res


def kernel(x, W_attn, b_attn, W_proj, b_proj):
    out, _ = run(x, W_attn, b_attn, W_proj, b_proj)
    return out


# revision 18
# speedup vs baseline: 1.1540x; 1.1540x over previous
"""Causal self-attention (B=2, T=2048, C=1024, H=16) on 8 trn2 NeuronCores.

Sharding: data-parallel over B (2) x tensor-parallel over head groups (4
groups of 4 heads).  core c -> batch c//4, head group c%4.  Each core
computes its 4 heads' qkv projection, attention, and the partial c_proj
contribution; the host sums the 4 tensor-parallel partials per batch
(the "all-reduce") and adds b_proj.

All matmuls run in bf16 (fp32r sustained at full rate trips the power
throttler, clamping the PE to 1.2 GHz).  Phases are interleaved per
t-block (qkv -> attention -> normalize -> c_proj) so the PE streams
back-to-back work; softmax exp is batched in [128,1024] pairs to
amortize ACT's per-instruction overhead; the softmax reciprocal runs on
the DVE.
"""

import sys
from contextlib import ExitStack

for _p in ("/opt/trn_rl_repo",):
    if _p not in sys.path:
        sys.path.insert(0, _p)

import ml_dtypes
import numpy as np

import concourse.bass as bass
import concourse.tile as tile
from concourse import mybir
from concourse.bass_utils import run_bass_kernel_spmd

F32 = mybir.dt.float32
F32R = mybir.dt.float32r
BF16 = mybir.dt.bfloat16
EXP = mybir.ActivationFunctionType.Exp

B, T, C = 2, 2048, 1024
H, D = 16, 64          # total heads, head dim
HL = 4                 # heads per core (local)
N_CORES = 8
QB = 512               # q block width
NTT = T // 128         # 16 t-tiles
NTB = T // QB          # 4 t-blocks
NC_C = C // 128        # 8 contraction tiles over C


def _merge(a, b):
    for k, v in b.items():
        if a.get(k, -1) < v:
            a[k] = v


def _reduce_matmul_waits(nc):
    """Sound transitive reduction of Matmult sync waits.

    Walrus rejects self-loading matmuls with >1 sync wait (the LDWEIGHTS
    struct has one wait slot).  Tile emits per-proc-minimal waits but does not
    track cross-proc transitivity, so e.g. a matmul recycling a PSUM slot
    waits on both the old writer (PE) and the old reader (ACT) even though the
    reader's wait already implies the writer finished.  We compute guarantee
    vector clocks (sem -> min value) for every sem increment and drop Matmult
    waits that are implied by the instruction's queue dispatch knowledge plus
    its remaining waits.
    """
    import bass_rust
    DMA_OPS = {"InstDMACopy", "InstDMATranspose"}
    dispatch = {}    # queue -> clock known at sequencer dispatch point
    done_prev = {}   # queue -> completion clock of previous engine inst
    sem_cum = {}     # sem -> cumulative inc
    sem_hist = {}    # sem -> list[(cum, prefix-merged clock)]
    n_dropped = 0

    def clock_at(sem, v):
        for cum, snap in sem_hist.get(sem, ()):
            if cum >= v:
                return snap
        return {}

    insts = [ins for bb in nc.main_func.blocks for ins in bb.instructions]
    for ins in insts:
        si = ins.sync_info
        q = str(getattr(ins, "engine", "?"))
        opc = type(ins).__name__
        dq = dispatch.setdefault(q, {})
        waits = list(si.on_wait) if si is not None else []
        wclocks = []
        for w in waits:
            wc = dict(clock_at(w.ant_name, w.wait_value))
            if wc.get(w.ant_name, -1) < w.wait_value:
                wc[w.ant_name] = w.wait_value
            wclocks.append(wc)

        if len(waits) > 1:
            # For serially-executing engines (DVE drains its pipe per op; ACT
            # and GpSimd likewise retire in order), the previous same-queue
            # instruction has fully completed by the time this one executes,
            # so its completion clock joins the implication base.  PE overlaps
            # matmul drains, and DMA lanes are async, so they only get
            # sequencer dispatch knowledge.
            serial = opc not in DMA_OPS and not q.endswith("PE")
            keep = set(range(len(waits)))
            order = sorted(
                range(len(waits)),
                key=lambda k: 0 if not waits[k].ant_name.startswith("DMA") else 1,
            )
            for k in order:
                if len(keep) <= 1:
                    break
                base = dict(dq)
                if serial:
                    _merge(base, done_prev.get(q, {}))
                for k2 in keep:
                    if k2 != k:
                        _merge(base, wclocks[k2])
                w = waits[k]
                if base.get(w.ant_name, -1) >= w.wait_value:
                    keep.discard(k)
            if len(keep) < len(waits):
                n_dropped += len(waits) - len(keep)
                ins.sync_info = bass_rust.SyncInfo(
                    on_wait=[waits[k] for k in sorted(keep)],
                    on_update=list(si.on_update),
                )

        for wc in wclocks:
            _merge(dq, wc)

        comp = dict(dq)
        if opc not in DMA_OPS:
            _merge(comp, done_prev.get(q, {}))

        ups = list(si.on_update) if si is not None else []
        for u in ups:
            if u.update_mode != "sem-inc":
                continue
            cum = sem_cum.get(u.ant_name, 0) + u.update_value
            sem_cum[u.ant_name] = cum
            hist = sem_hist.setdefault(u.ant_name, [])
            snap = dict(hist[-1][1]) if hist else {}
            _merge(snap, comp)
            snap[u.ant_name] = cum
            hist.append((cum, snap))
        if opc not in DMA_OPS:
            comp2 = dict(comp)
            for u in ups:
                if u.update_mode == "sem-inc":
                    comp2[u.ant_name] = max(
                        comp2.get(u.ant_name, 0), sem_cum[u.ant_name])
            done_prev[q] = comp2

    bad = [
        (ins.name, [(w.ant_name, w.wait_value) for w in ins.sync_info.on_wait])
        for ins in insts
        if type(ins).__name__ == "InstMatmult"
        and ins.sync_info is not None and len(ins.sync_info.on_wait) > 1
    ]
    if bad:
        raise RuntimeError(f"{len(bad)} matmuls still have >1 wait: {bad[:8]}")

    # This walrus accepts at most ONE sync wait per instruction struct.
    # Matmuls are handled above; for everything else, hoist the extra waits
    # onto standalone single-wait Drain carriers on the same queue (the
    # sequencer executes them in order, so the semantics are unchanged).
    wid = 0
    for bb in nc.main_func.blocks:
        out_list = []
        changed = False
        for ins in bb.instructions:
            si = ins.sync_info
            if (si is not None and len(si.on_wait) > 1
                    and type(ins).__name__ != "InstMatmult"):
                waits = list(si.on_wait)
                for w in waits[:-1]:
                    d = mybir.InstDrain(name=f"WSPLIT-{wid}", ins=[], outs=[])
                    wid += 1
                    d.engine = ins.engine
                    d.sync_info = bass_rust.SyncInfo(on_wait=[w], on_update=[])
                    try:
                        nc.register_instruction(d)
                    except Exception:
                        pass
                    out_list.append(d)
                ins.sync_info = bass_rust.SyncInfo(
                    on_wait=[waits[-1]], on_update=list(si.on_update))
                changed = True
            out_list.append(ins)
        if changed:
            bb.instructions = out_list

    # This neuronxcc's walrus rejects the raw-ISA EVENT_SEMAPHORE_RANGE_CLEAR
    # Tile emits as end-of-program semaphore hygiene ("ISA wrong length").
    # It has no sync side effects and only matters for back-to-back reuse of
    # the semaphore window inside one program, so drop it.
    for bb in nc.main_func.blocks:
        kept = [i for i in bb.instructions
                if not (type(i).__name__ == "InstISA"
                        and getattr(i, "op_name", "") ==
                        "EVENT_SEMAPHORE_RANGE_CLEAR")]
        if len(kept) != len(bb.instructions):
            bb.instructions = kept


def _build_nc(with_bias: bool = False) -> bass.Bass:
    nc = bass.Bass()

    xt = nc.declare_dram_parameter("xt", [C, T], BF16, False)
    wqk = nc.declare_dram_parameter("wqk", [C, 512], BF16, False)
    bqk = nc.declare_dram_parameter("bqk", [1, 512], BF16, False) if with_bias else None
    wv = nc.declare_dram_parameter("wv", [C, 256], BF16, False)
    bv = nc.declare_dram_parameter("bv", [1, 256], BF16, False) if with_bias else None
    wp4 = nc.declare_dram_parameter("wp4", [64, HL, C], BF16, False)
    tri = nc.declare_dram_parameter("tri", [128, 128], F32, False)
    out = nc.declare_dram_parameter("out", [T, C], F32, True)

    with tile.TileContext(nc) as tc, ExitStack() as ctx:
        consts = ctx.enter_context(tc.tile_pool(name="consts", bufs=1))
        wpool = ctx.enter_context(tc.tile_pool(name="wpool", bufs=1))
        big = ctx.enter_context(tc.tile_pool(name="big", bufs=1))
        xtp = ctx.enter_context(tc.tile_pool(name="xtp", bufs=2))
        ptp = ctx.enter_context(tc.tile_pool(name="ptp", bufs=3))
        yjp = ctx.enter_context(tc.tile_pool(name="yjp", bufs=4))
        otp = ctx.enter_context(tc.tile_pool(name="otp", bufs=2))
        small = ctx.enter_context(tc.tile_pool(name="small", bufs=2))
        bigps = ctx.enter_context(tc.tile_pool(name="bigps", bufs=3, space="PSUM"))
        psyp = ctx.enter_context(tc.tile_pool(name="psyp", bufs=1, space="PSUM"))
        psbp = ctx.enter_context(tc.tile_pool(name="psbp", bufs=1, space="PSUM"))

        # ---- constants ----
        tri_sb = consts.tile([128, 128], F32)   # additive: 0 if kk<=cc else -1e30
        nc.sync.dma_start(out=tri_sb, in_=tri[:])
        ones_b = consts.tile([1, 512], BF16)
        warm = consts.tile([1, 8], F32)

        if with_bias:
            bqk_sb = consts.tile([1, 512], BF16)
            nc.sync.dma_start(out=bqk_sb, in_=bqk[:])
            bv_sb = consts.tile([1, 256], BF16)
            nc.sync.dma_start(out=bv_sb, in_=bv[:])

        # ---- weights ----
        wqk_sb = wpool.tile([128, NC_C, 512], BF16)
        nc.sync.dma_start(out=wqk_sb, in_=wqk[:].rearrange("(c p) n -> p c n", p=128))
        wv_sb = wpool.tile([128, NC_C, 256], BF16)
        nc.sync.dma_start(out=wv_sb, in_=wv[:].rearrange("(c p) n -> p c n", p=128))
        wp_sb = wpool.tile([64, HL, C], BF16)
        nc.sync.dma_start(out=wp_sb, in_=wp4[:])

        # ---- persistent activations ----
        qkT = big.tile([128, 4, T], BF16)    # [qk ct, t]: rows = [q01|q23|k01|k23]*64
        v_sb = big.tile([128, NTT, HL, 65], BF16)   # v natural + ones column
        sums2 = big.tile([1, NTB * HL * 512], F32)   # raw softmax denoms, seg j*4+h
        recs = big.tile([1, NTB * HL * 512], BF16)   # 1/denominator

        # all constant memsets on DVE (single-proc tick, observed once by PE)
        nc.vector.memset(v_sb[:, :, :, 64:65], 1.0)
        nc.vector.memset(ones_b, 1.0)
        nc.vector.memset(warm, 0.0)
        nc.scalar.activation(out=warm, in_=warm, func=EXP)  # pre-load exp table

        # Self-loading matmuls support a single sync wait, so every real
        # matmul must have at most one un-observed dependency.  Funnel all
        # input-DMA completions through DVE copies, then observe DVE once on
        # the PE via a tiny "touch" matmul.  Touches write the psb bank and
        # get a DVE reader so their slot release is a DVE-only event.
        scr = consts.tile([1, 8], F32)
        funnel = [wqk_sb[0:1, 0, 0:1], wv_sb[0:1, 0, 0:1],
                  wp_sb[0:1, 0, 0:1], tri_sb[0:1, 0:1]]
        if with_bias:
            funnel += [bqk_sb[:, 0:1], bv_sb[:, 0:1]]
        for k, t_ap in enumerate(funnel):
            nc.vector.tensor_copy(out=scr[:, k:k + 1], in_=t_ap)
        scr2 = consts.tile([1, 1], F32)
        nc.vector.tensor_copy(out=scr2, in_=scr[:, 0:1])

        def touch(src_ap):
            sc = psbp.tile([1, 1], F32, tag="psb", name="sc")
            nc.tensor.matmul(out=sc, lhsT=src_ap, rhs=src_ap, start=True, stop=True)
            scd = small.tile([1, 1], F32, tag="scd", name="scd")
            nc.vector.tensor_copy(out=scd, in_=sc)

        touch(scr2)

        # ---- emission units: attention pairs interleaved with qkv/proj ----
        def load_x(tb):
            ts = slice(tb * QB, (tb + 1) * QB)
            xt_tb = xtp.tile([128, NC_C, QB], BF16)
            nc.sync.dma_start(
                out=xt_tb, in_=xt[:, ts].rearrange("(c p) n -> p c n", p=128)
            )
            return xt_tb

        def qkv_units(tb, xt_tb):
            ts = slice(tb * QB, (tb + 1) * QB)

            def touch_unit():
                xt_t = xtp.tile([1, 1], F32, tag="xt_t", name="xt_t")
                nc.vector.tensor_copy(out=xt_t, in_=xt_tb[0:1, 0, 0:1])
                touch(xt_t)

            def cp_unit(cp):
                # qk projection for ct pair cp into one 2-bank psum tile
                ps = bigps.tile([128, 1024], F32, tag="s")
                for sub in range(2):
                    ct = 2 * cp + sub
                    sl = slice(sub * 512, (sub + 1) * 512)
                    for c in range(NC_C):
                        nc.tensor.matmul(
                            out=ps[:, sl],
                            lhsT=wqk_sb[:, c, ct * 128:(ct + 1) * 128],
                            rhs=xt_tb[:, c, :],
                            start=(c == 0),
                            stop=(not with_bias and c == NC_C - 1),
                        )
                    if with_bias:
                        nc.tensor.matmul(  # + bias (outer product with ones)
                            out=ps[:, sl],
                            lhsT=bqk_sb[:, ct * 128:(ct + 1) * 128],
                            rhs=ones_b[:, 0:QB],
                            start=False,
                            stop=True,
                        )
                nc.scalar.copy(
                    out=qkT[:, 2 * cp:2 * cp + 2, ts],
                    in_=ps[:].rearrange("p (c n) -> p c n", c=2),
                )

            def v_unit(half):
                # v projection for two t-tiles into a 1-bank psum tile
                ps = bigps.tile([128, 512], F32, tag="s", name="psv")
                for sub in range(2):
                    t4 = 2 * half + sub
                    sl = slice(sub * 256, (sub + 1) * 256)
                    for c in range(NC_C):
                        nc.tensor.matmul(
                            out=ps[:, sl],
                            lhsT=xt_tb[:, c, t4 * 128:(t4 + 1) * 128],
                            rhs=wv_sb[:, c, :],
                            start=(c == 0),
                            stop=(not with_bias and c == NC_C - 1),
                        )
                    if with_bias:
                        nc.tensor.matmul(
                            out=ps[:, sl],
                            lhsT=ones_b[:, 0:128],
                            rhs=bv_sb[:],
                            start=False,
                            stop=True,
                        )
                tt = tb * 4 + 2 * half
                nc.scalar.copy(
                    out=v_sb[:, tt:tt + 2, :, 0:64],
                    in_=ps[:].rearrange("p (t h d) -> p t h d", t=2, h=HL),
                )

            return [touch_unit,
                    lambda: cp_unit(0), lambda: cp_unit(1),
                    lambda: v_unit(0), lambda: v_unit(1)]

        def norm_broadcast(j, h, yjs):
            # psb = ones x (1/sums[j,h]); emitted at least one head late so
            # the PE FIFO never stalls on the DVE reciprocal chain.
            idx = (j * HL + h) * 512
            psb = psbp.tile([64, 512], F32, tag="psb", name="psb")
            nc.tensor.matmul(
                out=psb, lhsT=ones_b[:, 0:64],
                rhs=recs[:, idx:idx + 512], start=True, stop=True,
            )
            nc.vector.tensor_mul(out=yjs[h], in0=yjs[h], in1=psb)

        def attn(j, yjs, filler):
            # Two-deep software pipeline: S+exp of pair k+2 are emitted before
            # PV of pair k, so the ACT exp stream runs 1-2 pairs ahead of the
            # PE's PV consumption.  Filler units (qkv of a later block / proj
            # of an earlier one) slot in before a PV with zero exp-wait, and
            # their ACT copies are absorbed by the standing exp lead.
            qs = slice(j * QB, (j + 1) * QB)
            nk = 4 * j + 4
            pairs = [(h, m) for h in range(HL) for m in range(nk // 2)]
            psys, pts = {}, {}
            interval = max(1, len(pairs) // (len(filler) + 1))

            def emit_s(k):
                h, m = pairs[k]
                ct_q, ct_k, base = h // 2, 2 + h // 2, (h % 2) * 64
                rows = slice(base, base + 64)
                ps = bigps.tile([128, 1024], F32, tag="s")
                os = [max(0, 128 * (2 * m + sub - 4 * j)) for sub in range(2)]
                diag = 2 * m >= 4 * j
                for sub in range(2):
                    i = 2 * m + sub
                    # trim diagonal S tiles to the causal columns
                    nc.tensor.matmul(
                        out=ps[:, sub * 512 + os[sub]:(sub + 1) * 512],
                        lhsT=qkT[rows, ct_k, i * 128:(i + 1) * 128],
                        rhs=qkT[rows, ct_q, qs][:, os[sub]:512],
                        start=True,
                        stop=True,
                    )
                    if diag:
                        nc.vector.tensor_add(
                            out=ps[:, sub * 512 + os[sub]:
                                   sub * 512 + os[sub] + 128],
                            in0=ps[:, sub * 512 + os[sub]:
                                    sub * 512 + os[sub] + 128],
                            in1=tri_sb,
                        )
                p_t = ptp.tile([128, 1024], BF16, tag="p")
                pts[k] = (p_t, os)
                if diag:
                    # split exp per sub-tile: psum cols below os are unwritten
                    for sub in range(2):
                        sl = slice(sub * 512 + os[sub], (sub + 1) * 512)
                        nc.scalar.activation(
                            out=p_t[:, sl], in_=ps[:, sl], func=EXP,
                            scale=0.125,
                        )
                else:
                    nc.scalar.activation(out=p_t, in_=ps, func=EXP, scale=0.125)

            def emit_pv(k):
                h, m = pairs[k]
                if h not in psys:
                    psys[h] = psyp.tile([65, 512], F32, tag="psy", name="psy")
                psy = psys[h]
                p_t, os = pts.pop(k)
                for sub in range(2):
                    i = 2 * m + sub
                    o = os[sub]
                    nc.tensor.matmul(
                        out=psy[:, o:512],
                        lhsT=v_sb[:, i, h, :],
                        rhs=p_t[:, sub * 512 + o:(sub + 1) * 512],
                        start=(i == 0),
                        stop=(i == nk - 1),
                    )
                if m == nk // 2 - 1:   # head done: sums, 1/sums, yj copy-out
                    del psys[h]
                    yj = yjp.tile([64, 512], BF16, tag=f"yj{h}", name=f"yj{h}")
                    yjs[h] = yj
                    idx = (j * HL + h) * 512
                    nc.vector.tensor_copy(
                        out=sums2[:, idx:idx + 512], in_=psy[64:65, :]
                    )
                    with nc.allow_low_precision(reason="softmax 1/sum bf16"):
                        nc.vector.reciprocal(
                            out=recs[:, idx:idx + 512],
                            in_=sums2[:, idx:idx + 512],
                        )
                    nc.vector.tensor_copy(out=yj, in_=psy[0:64, :])
                    if h >= 1:
                        norm_broadcast(j, h - 1, yjs)

            emit_s(0)
            if len(pairs) > 1:
                emit_s(1)
            for k in range(len(pairs)):
                if k % interval == interval - 1 and filler:
                    filler.pop(0)()   # dense PE work; exp lead absorbs copies
                emit_pv(k)
                if k + 2 < len(pairs):
                    emit_s(k + 2)
            while filler:
                filler.pop(0)()

        def proj_units(tb, yjs):
            def t4_unit(t4):
                pso = bigps.tile([128, 1024], F32, tag="s")
                for co in range(2):
                    for h in range(HL):
                        nc.tensor.matmul(
                            out=pso[:, co * 512:(co + 1) * 512],
                            lhsT=yjs[h][:, t4 * 128:(t4 + 1) * 128],
                            rhs=wp_sb[:, h, co * 512:(co + 1) * 512],
                            start=(h == 0),
                            stop=(h == HL - 1),
                        )
                ot = otp.tile([128, C], F32, tag="ot")
                nc.scalar.copy(out=ot[:, 0:512], in_=pso[:, 0:512])
                nc.vector.tensor_copy(out=ot[:, 512:1024], in_=pso[:, 512:1024])
                t0 = tb * QB + t4 * 128
                nc.sync.dma_start(out=out[t0:t0 + 128, :], in_=ot)

            return [lambda t4=t4: t4_unit(t4) for t4 in range(4)]

        # Filler schedule (proj deferred up to two blocks so the long late
        # attention phases get enough dense PE work):
        #   attn(0): qkv(1)   attn(1): qkv(2)   attn(2): qkv(3)+proj(0)
        #   attn(3): proj(1)+proj(2)   tail: proj(3)
        xt_cur = load_x(0)
        for u in qkv_units(0, xt_cur):
            u()
        yjs_all = {}
        for tb in range(NTB):
            filler = []
            if tb + 1 < NTB:
                xt_nxt = load_x(tb + 1)
                filler += qkv_units(tb + 1, xt_nxt)
            if tb == 2:
                filler += proj_units(0, yjs_all[0])
            elif tb == 3:
                filler += proj_units(1, yjs_all[1])
                filler += proj_units(2, yjs_all[2])
            yjs = {}
            attn(tb, yjs, filler)
            norm_broadcast(tb, HL - 1, yjs)
            yjs_all[tb] = yjs
        for u in proj_units(NTB - 1, yjs_all[NTB - 1]):
            u()

    _reduce_matmul_waits(nc)
    return nc


_NC_CACHE = {}


def _get_nc(with_bias: bool = False):
    if with_bias not in _NC_CACHE:
        _NC_CACHE[with_bias] = _build_nc(with_bias)
    return _NC_CACHE[with_bias]


def make_in_maps(x, W_attn, b_attn, W_proj, b_proj):
    bf16 = ml_dtypes.bfloat16
    x = np.asarray(x, dtype=np.float32)
    W_attn = np.asarray(W_attn, dtype=np.float32)
    b_attn = np.asarray(b_attn, dtype=np.float32)
    W_proj = np.asarray(W_proj, dtype=np.float32)
    kk, cc = np.meshgrid(np.arange(128), np.arange(128), indexing="ij")
    tri = np.where(kk <= cc, 0.0, -1e30).astype(np.float32)
    in_maps = []
    for core in range(N_CORES):
        b, hg = core // 4, core % 4
        qc = slice(hg * 256, (hg + 1) * 256)
        kc = slice(C + hg * 256, C + (hg + 1) * 256)
        vc = slice(2 * C + hg * 256, 2 * C + (hg + 1) * 256)
        in_maps.append({
            "xt": np.ascontiguousarray(x[b].T).astype(bf16),
            "wqk": np.ascontiguousarray(
                np.concatenate([W_attn[:, qc], W_attn[:, kc]], axis=1)
            ).astype(bf16),
            "bqk": np.concatenate([b_attn[qc], b_attn[kc]]).reshape(1, 512)
            .astype(bf16),
            "wv": np.ascontiguousarray(W_attn[:, vc]).astype(bf16),
            "bv": b_attn[vc].reshape(1, 256).astype(bf16),
            "wp4": np.ascontiguousarray(
                W_proj[hg * 256:(hg + 1) * 256].reshape(HL, 64, C)
                .transpose(1, 0, 2)).astype(bf16),
            "tri": tri,
        })
    return in_maps


def gather(results, b_proj):
    b_proj = np.asarray(b_proj, dtype=np.float32)
    out = np.empty((B, T, C), dtype=np.float32)
    for b in range(B):
        acc = results[4 * b]["out"].astype(np.float32)
        for g in range(1, 4):
            acc = acc + results[4 * b + g]["out"]
        out[b] = acc + b_proj
    return out


def run(x, W_attn, b_attn, W_proj, b_proj, trace=False):
    with_bias = bool(np.any(np.asarray(b_attn)))
    nc = _get_nc(with_bias=with_bias)
    in_maps = make_in_maps(x, W_attn, b_attn, W_proj, b_proj)
    if not with_bias:
        for m in in_maps:
            m.pop("bqk", None)
            m.pop("bv", None)
    res = run_bass_kernel_spmd(nc, in_maps, list(range(N_CORES)), trace=trace)
    return gather(res.results, b_proj), res


def kernel(x, W_attn, b_attn, W_proj, b_proj):
    out, _ = run(x, W_attn, b_attn, W_proj, b_proj)
    return out
